# revision 3
# baseline (speedup 1.0000x reference)
"""Trainium2 Bass kernel for the DigitConvolutionalModel problem.

Math: out = relu(conv3x3(x) @ fc1_w.T + fc1_b) @ fc2_w.T + fc2_b
The 3x3 valid conv followed by a dense layer composes into a single
linear map, so conv_w and fc1_w are folded on the host into one
W1eff [128, 784] matrix. The device then runs two matmuls + bias/relu.

Sharding: pure data parallelism — batch split across 8 cores.
Each core's x shard is staged transposed ([784, 8192]) so the
contraction dim lands on SBUF partitions with contiguous DMA.

Precision: fc1 runs as a compensated fp16 product — x and W1eff are
each split into hi+lo fp16 pairs (same total bytes over HBM as f32)
and combined as xh@Wh + xh@Wl + xl@Wh into the f32 PSUM, giving
near-f32 accuracy at fp16 matmul throughput. fc2 (tiny K=128) runs in
plain f32.
"""

import numpy as np

import concourse.bacc as bacc
import concourse.mybir as mybir
import concourse.tile as tile
from concourse.bass_utils import run_bass_kernel_spmd

N_CORES = 8
B = 65536
B_LOCAL = B // N_CORES  # 8192
K = 784                 # input features (28*28)
M1 = 128                # fc1 out
M2 = 10                 # fc2 out
KCS = [128] * 6 + [16]  # contraction chunks of 784

F32 = mybir.dt.float32
BF16 = mybir.dt.bfloat16
FP16 = mybir.dt.float16

# "fp16x2": compensated fp16 split (near-f32 accuracy)
# "fp16" / "bf16": single-pass low precision (x cast during DMA)
# "f32": plain fp32 matmuls (4 cycles/row)
MODE = "fp16x2"
BT = 2048               # batch tile per DMA
NS = 512                # matmul moving-dim subtile (one PSUM bank)

_cache = {}


def _build_nc(mode=MODE, bt=BT, ns=NS):
    nc = bacc.Bacc("TRN2", target_bir_lowering=False, debug=False,
                   num_devices=N_CORES)
    split = mode == "fp16x2"
    mm_dt = {"fp16x2": FP16, "fp16": FP16, "bf16": BF16, "f32": F32}[mode]
    x_dt = FP16 if split else F32

    xh_d = nc.dram_tensor("x_h", [K, B_LOCAL], x_dt, kind="ExternalInput")
    if split:
        xl_d = nc.dram_tensor("x_l", [K, B_LOCAL], FP16, kind="ExternalInput")
    w1h_d = nc.dram_tensor("w1t_h", [K, M1], F32 if not split else FP16,
                           kind="ExternalInput")
    if split:
        w1l_d = nc.dram_tensor("w1t_l", [K, M1], FP16, kind="ExternalInput")
    b1_d = nc.dram_tensor("b1", [M1, 1], F32, kind="ExternalInput")
    w2_d = nc.dram_tensor("w2t", [M1, M2], F32, kind="ExternalInput")
    b2_d = nc.dram_tensor("b2", [M2, 1], F32, kind="ExternalInput")
    z_d = nc.dram_tensor("z_t", [M2, B_LOCAL], F32, kind="ExternalOutput")

    mm2_dt = F32 if (split or mode == "f32") else mm_dt

    with tile.TileContext(nc) as tc:
        with (
            tc.tile_pool(name="static", bufs=1) as sp,
            tc.tile_pool(name="xp", bufs=2) as xp,
            tc.tile_pool(name="hp", bufs=3) as hp,
            tc.tile_pool(name="zp", bufs=3) as zp,
            tc.tile_pool(name="pp1", bufs=2, space="PSUM") as pp1,
            tc.tile_pool(name="pp2", bufs=2, space="PSUM") as pp2,
        ):
            w1hs, w1ls = [], []
            off = 0
            for kc, ks in enumerate(KCS):
                wt = sp.tile([ks, M1], mm_dt, tag=f"w1h_{kc}")
                nc.gpsimd.dma_start(wt[:], w1h_d[off:off + ks, :])
                w1hs.append(wt)
                if split:
                    wl = sp.tile([ks, M1], FP16, tag=f"w1l_{kc}")
                    nc.gpsimd.dma_start(wl[:], w1l_d[off:off + ks, :])
                    w1ls.append(wl)
                off += ks
            w2t = sp.tile([M1, M2], mm2_dt, tag="w2")
            nc.gpsimd.dma_start(w2t[:], w2_d[:])
            b1t = sp.tile([M1, 1], F32, tag="b1")
            nc.gpsimd.dma_start(b1t[:], b1_d[:])
            b2t = sp.tile([M2, 1], F32, tag="b2")
            nc.gpsimd.dma_start(b2t[:], b2_d[:])

            for bt_i in range(B_LOCAL // bt):
                xhs, xls = [], []
                off = 0
                for kc, ks in enumerate(KCS):
                    xt = xp.tile([ks, bt], mm_dt, tag=f"xh{kc}")
                    nc.gpsimd.dma_start(
                        xt[:], xh_d[off:off + ks, bt_i * bt:(bt_i + 1) * bt])
                    xhs.append(xt)
                    if split:
                        xt2 = xp.tile([ks, bt], FP16, tag=f"xl{kc}")
                        nc.gpsimd.dma_start(
                            xt2[:],
                            xl_d[off:off + ks, bt_i * bt:(bt_i + 1) * bt])
                        xls.append(xt2)
                    off += ks
                for ns_i in range(bt // ns):
                    sl = slice(ns_i * ns, (ns_i + 1) * ns)
                    ps1 = pp1.tile([M1, ns], F32, tag="ps1")
                    nkc = len(KCS)
                    if split:
                        pairs = (
                            [(w1hs[kc], xhs[kc]) for kc in range(nkc)]
                            + [(w1ls[kc], xhs[kc]) for kc in range(nkc)]
                            + [(w1hs[kc], xls[kc]) for kc in range(nkc)]
                        )
                    else:
                        pairs = [(w1hs[kc], xhs[kc]) for kc in range(nkc)]
                    for i, (wt, xt) in enumerate(pairs):
                        nc.tensor.matmul(
                            ps1[:], wt[:], xt[:, sl],
                            start=(i == 0), stop=(i == len(pairs) - 1))
                    h = hp.tile([M1, ns], mm2_dt, tag="h")
                    nc.scalar.activation(
                        h[:], ps1[:], mybir.ActivationFunctionType.Relu,
                        bias=b1t[:])
                    ps2 = pp2.tile([M2, ns], F32, tag="ps2")
                    nc.tensor.matmul(
                        ps2[:], w2t[:], h[:], start=True, stop=True)
                    zt = zp.tile([M2, ns], F32, tag="z")
                    nc.vector.tensor_scalar_add(zt[:], ps2[:], b2t[:])
                    nc.gpsimd.dma_start(
                        z_d[:, bt_i * bt + ns_i * ns:
                            bt_i * bt + (ns_i + 1) * ns],
                        zt[:])
    nc.compile()
    return nc


def _fold_weights(conv_w, fc1_w):
    """Fold 3x3 valid cross-correlation + fc1 into one [128, 784] matrix."""
    cw = np.asarray(conv_w, np.float64)
    f1 = np.asarray(fc1_w, np.float64).reshape(M1, 26, 26)
    W = np.zeros((M1, 28, 28), np.float64)
    for di in range(3):
        for dj in range(3):
            W[:, di:di + 26, dj:dj + 26] += cw[di, dj] * f1
    return W.reshape(M1, K).astype(np.float32)


def _split16(a):
    hi = a.astype(np.float16)
    lo = (a.astype(np.float32) - hi.astype(np.float32)).astype(np.float16)
    return hi, lo


def kernel(x, conv_w, fc1_w, fc1_b, fc2_w, fc2_b):
    if "nc" not in _cache:
        _cache["nc"] = _build_nc()
    nc = _cache["nc"]

    w1t = np.ascontiguousarray(_fold_weights(conv_w, fc1_w).T)  # [784, 128]
    b1 = np.ascontiguousarray(np.asarray(fc1_b, np.float32).reshape(M1, 1))
    w2t = np.ascontiguousarray(np.asarray(fc2_w, np.float32).T)  # [128, 10]
    b2 = np.ascontiguousarray(np.asarray(fc2_b, np.float32).reshape(M2, 1))
    x = np.asarray(x, np.float32)

    split = MODE == "fp16x2"
    if split:
        w1t_h, w1t_l = _split16(w1t)
    in_maps = []
    for c in range(N_CORES):
        xs = np.ascontiguousarray(x[c * B_LOCAL:(c + 1) * B_LOCAL].T)
        if split:
            xh, xl = _split16(xs)
            m = {"x_h": xh, "x_l": xl, "w1t_h": w1t_h, "w1t_l": w1t_l}
        else:
            m = {"x_h": xs, "w1t_h": w1t}
        m.update({"b1": b1, "w2t": w2t, "b2": b2})
        in_maps.append(m)
    res = run_bass_kernel_spmd(nc, in_maps, list(range(N_CORES)))
    outs = [res.results[c]["z_t"].T for c in range(N_CORES)]
    return np.ascontiguousarray(np.concatenate(outs, axis=0), dtype=np.float32)


# revision 11
# speedup vs baseline: 1.2369x; 1.2369x over previous
"""Trainium2 Bass kernel for the DigitConvolutionalModel problem.

Math: out = relu(conv3x3(x) @ fc1_w.T + fc1_b) @ fc2_w.T + fc2_b
The 3x3 valid conv followed by a dense layer composes into a single
linear map, so conv_w and fc1_w are folded on the host into one
W1eff [128, 784] matrix. The device then runs two matmuls + bias/relu.

Sharding: pure data parallelism — batch split across 8 cores.
Each core's x shard is staged transposed ([784, 8192]) so the
contraction dim lands on SBUF partitions with contiguous DMA.

Precision: fc1 runs as a compensated fp16 product — x and W1eff are
each split into hi+lo fp16 pairs (same total bytes over HBM as f32)
and combined as xh@Wh + xh@Wl + xl@Wh into the f32 PSUM, giving
near-f32 accuracy at fp16 matmul throughput. The three 16-row K-tail
products are packed into one 48-row chunk so every matmul contracts
a full-ish partition block. fc2 (tiny K=128) runs in plain f32.
"""

import numpy as np

import concourse.bacc as bacc
import concourse.mybir as mybir
import concourse.tile as tile
from concourse.bass_utils import run_bass_kernel_spmd

N_CORES = 8
B = 65536
B_LOCAL = B // N_CORES  # 8192
K = 784                 # input features (28*28)
KM = 768                # main K rows (6 chunks of 128)
KT = 48                 # packed tail rows: [xh_t; xh_t; xl_t] x 16
M1 = 128                # fc1 out
M2 = 10                 # fc2 out
NKC = 6                 # main K chunks

F32 = mybir.dt.float32
FP16 = mybir.dt.float16

MODE = "fp16x2"
BT = 2048               # batch tile per DMA
NS = 512                # matmul moving-dim subtile (one PSUM bank)

_cache = {}


def _bt_schedule(total=B_LOCAL, ns=NS, bt=1024):
    """Uniform tiles: DMA delivery and PE consumption rates are nearly
    equal, so any size jump starves one side."""
    assert total % bt == 0 and bt % ns == 0
    return [bt] * (total // bt)


def _build_nc(mode=MODE, bt=BT, ns=NS):
    assert mode == "fp16x2"
    nc = bacc.Bacc("TRN2", target_bir_lowering=False, debug=False,
                   num_devices=N_CORES)

    xh_d = nc.dram_tensor("x_h", [KM, B_LOCAL], FP16, kind="ExternalInput")
    xl_d = nc.dram_tensor("x_l", [KM, B_LOCAL], FP16, kind="ExternalInput")
    xt_d = nc.dram_tensor("x_tail", [KT, B_LOCAL], FP16, kind="ExternalInput")
    # All matmul weights packed as column blocks of one [128, 1684] tensor:
    # cols 0:768 = 6 Wh chunks, 768:1536 = 6 Wl chunks, 1536:1664 = packed
    # tail (rows 0:48), 1664:1674 = W2h, 1674:1684 = W2l.
    wall_d = nc.dram_tensor("w_all", [128, 1664], FP16, kind="ExternalInput")
    # f32 pack: col 0 = b1, col 1 rows 0:10 = b2, cols 2:12 = W2 (f32)
    bias_d = nc.dram_tensor("biases", [M1, 12], F32, kind="ExternalInput")
    z_d = nc.dram_tensor("z_t", [M2, B_LOCAL], F32, kind="ExternalOutput")

    with tile.TileContext(nc) as tc:
        with (
            tc.tile_pool(name="static", bufs=1) as sp,
            tc.tile_pool(name="xp", bufs=3) as xp,
            tc.tile_pool(name="hp", bufs=3) as hp,
            tc.tile_pool(name="zp", bufs=2) as zp,
            tc.tile_pool(name="pp1", bufs=4, space="PSUM") as pp1,
            tc.tile_pool(name="pp2", bufs=2, space="PSUM") as pp2,
        ):
            # One DMA for all weights, one for both biases, on the
            # (otherwise idle) GPSIMD SWDGE path — off the HWDGE x rings.
            wall = sp.tile([128, 1664], FP16, tag="w_all")
            nc.gpsimd.dma_start(wall[:], wall_d[:])
            bias = sp.tile([M1, 12], F32, tag="biases")
            nc.gpsimd.dma_start(bias[:], bias_d[:])
            w1hs = [wall[:, kc * 128:(kc + 1) * 128] for kc in range(NKC)]
            w1ls = [wall[:, 768 + kc * 128: 768 + (kc + 1) * 128]
                    for kc in range(NKC)]
            wtl = wall[0:KT, 1536:1664]
            b1t = bias[:, 0:1]
            b2t = bias[0:M2, 1:2]
            w2t = bias[:, 2:12]

            bts = _bt_schedule(B_LOCAL, ns)
            offs = [sum(bts[:i]) for i in range(len(bts))]
            xtiles = [None] * len(bts)
            # [768, B] viewed as [128 partitions, 6 chunks, B] so one SWDGE
            # DMA moves all six k-chunks of a batch tile.
            xh_v = xh_d.rearrange("(c p) b -> p c b", p=128)
            xl_v = xl_d.rearrange("(c p) b -> p c b", p=128)

            def load_bt(i):
                """Issue bt i's x DMAs (3 fused SWDGE transfers)."""
                btc = bts[i]
                bsl = slice(offs[i], offs[i] + btc)
                xh_all = xp.tile([128, NKC, btc], FP16, tag="xh")
                nc.gpsimd.dma_start(xh_all[:], xh_v[:, :, bsl])
                xl_all = xp.tile([128, NKC, btc], FP16, tag="xl")
                nc.gpsimd.dma_start(xl_all[:], xl_v[:, :, bsl])
                xtl = xp.tile([KT, btc], FP16, tag="xtail")
                nc.gpsimd.dma_start(xtl[:], xt_d[:, bsl])
                xhs = [xh_all[:, kc, :] for kc in range(NKC)]
                xls = [xl_all[:, kc, :] for kc in range(NKC)]
                xtiles[i] = (xhs, xls, xtl)

            load_bt(0)
            load_bt(1)
            for bt_i, btc in enumerate(bts):
                if bt_i + 2 < len(bts):
                    load_bt(bt_i + 2)  # prefetch two batch tiles ahead
                bsl = slice(offs[bt_i], offs[bt_i] + btc)
                xhs, xls, xtl = xtiles[bt_i]
                zt = zp.tile([M2, btc], F32, tag="z")
                for ns_i in range(btc // ns):
                    sl = slice(ns_i * ns, (ns_i + 1) * ns)
                    ps1 = pp1.tile([M1, ns], F32, tag="ps1")
                    pairs = (
                        [(w1hs[kc], xhs[kc]) for kc in range(NKC)]
                        + [(w1ls[kc], xhs[kc]) for kc in range(NKC)]
                        + [(w1hs[kc], xls[kc]) for kc in range(NKC)]
                        + [(wtl, xtl)]
                    )
                    for i, (wt, xt) in enumerate(pairs):
                        nc.tensor.matmul(
                            ps1[:], wt, xt[:, sl],
                            start=(i == 0), stop=(i == len(pairs) - 1))
                    h = hp.tile([M1, ns], F32, tag="h")
                    nc.scalar.activation(
                        h[:], ps1[:], mybir.ActivationFunctionType.Relu,
                        bias=b1t)
                    ps2 = pp2.tile([M2, ns], F32, tag="ps2")
                    nc.tensor.matmul(
                        ps2[:], w2t, h[:], start=True, stop=True)
                    nc.vector.tensor_scalar_add(zt[:, sl], ps2[:], b2t)
                nc.gpsimd.dma_start(z_d[:, bsl], zt[:])
    nc.compile()
    return nc


def _fold_weights(conv_w, fc1_w):
    """Fold 3x3 valid cross-correlation + fc1 into one [128, 784] matrix."""
    cw = np.asarray(conv_w, np.float64)
    f1 = np.asarray(fc1_w, np.float64).reshape(M1, 26, 26)
    W = np.zeros((M1, 28, 28), np.float64)
    for di in range(3):
        for dj in range(3):
            W[:, di:di + 26, dj:dj + 26] += cw[di, dj] * f1
    return W.reshape(M1, K).astype(np.float32)


def _split16(a):
    hi = a.astype(np.float16)
    lo = (a.astype(np.float32) - hi.astype(np.float32)).astype(np.float16)
    return hi, lo


def kernel(x, conv_w, fc1_w, fc1_b, fc2_w, fc2_b):
    if "nc" not in _cache:
        _cache["nc"] = _build_nc()
    nc = _cache["nc"]

    w1t = np.ascontiguousarray(_fold_weights(conv_w, fc1_w).T)  # [784, 128]
    w1t_h, w1t_l = _split16(w1t)
    w_tail = np.vstack([w1t_h[KM:], w1t_l[KM:], w1t_h[KM:]])  # [48, 128]
    w2t = np.asarray(fc2_w, np.float32).T  # [128, 10]
    w_all = np.zeros((128, 1664), np.float16)
    for kc in range(NKC):
        w_all[:, kc * 128:(kc + 1) * 128] = w1t_h[kc * 128:(kc + 1) * 128, :]
        w_all[:, 768 + kc * 128: 768 + (kc + 1) * 128] = \
            w1t_l[kc * 128:(kc + 1) * 128, :]
    w_all[0:KT, 1536:1664] = w_tail
    w_all = np.ascontiguousarray(w_all)
    biases = np.zeros((M1, 12), np.float32)
    biases[:, 0] = np.asarray(fc1_b, np.float32)
    biases[0:M2, 1] = np.asarray(fc2_b, np.float32)
    biases[:, 2:12] = w2t
    x = np.asarray(x, np.float32)

    in_maps = []
    for c in range(N_CORES):
        xs = np.ascontiguousarray(x[c * B_LOCAL:(c + 1) * B_LOCAL].T)
        xh, xl = _split16(xs)
        # tail rows ordered to match w_tail: [xh_t (vs Wh), xh_t (vs Wl),
        # xl_t (vs Wh)]
        x_tail = np.ascontiguousarray(
            np.vstack([xh[KM:], xh[KM:], xl[KM:]]))  # [48, B_LOCAL]
        in_maps.append({
            "x_h": np.ascontiguousarray(xh[:KM]),
            "x_l": np.ascontiguousarray(xl[:KM]),
            "x_tail": x_tail,
            "w_all": w_all, "biases": biases,
        })
    res = run_bass_kernel_spmd(nc, in_maps, list(range(N_CORES)))
    outs = [res.results[c]["z_t"].T for c in range(N_CORES)]
    return np.ascontiguousarray(np.concatenate(outs, axis=0), dtype=np.float32)


# revision 17
# speedup vs baseline: 1.3096x; 1.0588x over previous
"""Trainium2 Bass kernel for the DigitConvolutionalModel problem.

Math: out = relu(conv3x3(x) @ fc1_w.T + fc1_b) @ fc2_w.T + fc2_b
The 3x3 valid conv followed by a dense layer composes into a single
linear map, so conv_w and fc1_w are folded on the host into one
W1eff [128, 784] matrix. The device then runs two matmuls + bias/relu.

Sharding: pure data parallelism — batch split across 8 cores.
Each core's x shard is staged transposed ([784, 8192]) so the
contraction dim lands on SBUF partitions with contiguous DMA.

Precision: fc1 runs as a compensated fp16 product — x and W1eff are
each split into hi+lo fp16 pairs (same total bytes over HBM as f32)
and combined as xh@Wh + xh@Wl + xl@Wh into the f32 PSUM, giving
near-f32 accuracy at fp16 matmul throughput. The three 16-row K-tail
products are packed into one 48-row chunk so every matmul contracts
a full-ish partition block. fc2 (tiny K=128) runs in plain f32.
"""

import numpy as np

import concourse.bacc as bacc
import concourse.mybir as mybir
import concourse.tile as tile
from concourse.bass_utils import run_bass_kernel_spmd

N_CORES = 8
B = 65536
B_LOCAL = B // N_CORES  # 8192
K = 784                 # input features (28*28)
KM = 768                # main K rows (6 chunks of 128)
KT = 48                 # packed tail rows: [xh_t; xh_t; xl_t] x 16
M1 = 128                # fc1 out
M2 = 10                 # fc2 out
NKC = 6                 # main K chunks

F32 = mybir.dt.float32
FP16 = mybir.dt.float16

MODE = "fp16x2"
BT = 2048               # batch tile per DMA
NS = 512                # matmul moving-dim subtile (one PSUM bank)

_cache = {}


def _bt_schedule(total=B_LOCAL, ns=NS, bt=1024):
    """Uniform tiles: DMA delivery and PE consumption rates are nearly
    equal, so any size jump starves one side."""
    assert total % bt == 0 and bt % ns == 0
    return [bt] * (total // bt)


def _build_nc(mode=MODE, bt=BT, ns=NS):
    assert mode == "fp16x2"
    nc = bacc.Bacc("TRN2", target_bir_lowering=False, debug=False,
                   num_devices=N_CORES)

    xh_d = nc.dram_tensor("x_h", [KM, B_LOCAL], FP16, kind="ExternalInput")
    xl_d = nc.dram_tensor("x_l", [KM, B_LOCAL], FP16, kind="ExternalInput")
    xt_d = nc.dram_tensor("x_tail", [KT, B_LOCAL], FP16, kind="ExternalInput")
    # All matmul weights packed as column blocks of one [128, 1684] tensor:
    # cols 0:768 = 6 Wh chunks, 768:1536 = 6 Wl chunks, 1536:1664 = packed
    # tail (rows 0:48), 1664:1674 = W2h, 1674:1684 = W2l.
    wall_d = nc.dram_tensor("w_all", [128, 1664], FP16, kind="ExternalInput")
    # f32 pack: col 0 = b1, col 1 rows 0:10 = b2, cols 2:12 = W2 (f32)
    bias_d = nc.dram_tensor("biases", [M1, 12], F32, kind="ExternalInput")
    z_d = nc.dram_tensor("z_t", [M2, B_LOCAL], F32, kind="ExternalOutput")

    with tile.TileContext(nc) as tc:
        with (
            tc.tile_pool(name="static", bufs=1) as sp,
            tc.tile_pool(name="xp", bufs=3) as xp,
            tc.tile_pool(name="hp", bufs=4) as hp,
            tc.tile_pool(name="zp", bufs=3) as zp,
            tc.tile_pool(name="pp1", bufs=4, space="PSUM") as pp1,
            tc.tile_pool(name="pp2", bufs=2, space="PSUM") as pp2,
        ):
            # One DMA for all weights, one for both biases, on the
            # (otherwise idle) GPSIMD SWDGE path — off the HWDGE x rings.
            wall = sp.tile([128, 1664], FP16, tag="w_all")
            nc.gpsimd.dma_start(wall[:], wall_d[:])
            bias = sp.tile([M1, 12], F32, tag="biases")
            nc.gpsimd.dma_start(bias[:], bias_d[:])
            w1hs = [wall[:, kc * 128:(kc + 1) * 128] for kc in range(NKC)]
            w1ls = [wall[:, 768 + kc * 128: 768 + (kc + 1) * 128]
                    for kc in range(NKC)]
            wtl = wall[0:KT, 1536:1664]
            b1t = bias[:, 0:1]
            b2t = bias[0:M2, 1:2]
            w2t = bias[:, 2:12]

            bts = _bt_schedule(B_LOCAL, ns)
            offs = [sum(bts[:i]) for i in range(len(bts))]
            xtiles = [None] * len(bts)
            # [768, B] viewed as [128 partitions, 6 chunks, B] so one SWDGE
            # DMA moves all six k-chunks of a batch tile.
            xh_v = xh_d.rearrange("(c p) b -> p c b", p=128)
            xl_v = xl_d.rearrange("(c p) b -> p c b", p=128)

            def load_bt(i):
                """Issue bt i's x DMAs (3 fused SWDGE transfers)."""
                btc = bts[i]
                bsl = slice(offs[i], offs[i] + btc)
                xh_all = xp.tile([128, NKC, btc], FP16, tag="xh")
                nc.gpsimd.dma_start(xh_all[:], xh_v[:, :, bsl])
                xl_all = xp.tile([128, NKC, btc], FP16, tag="xl")
                nc.gpsimd.dma_start(xl_all[:], xl_v[:, :, bsl])
                xtl = xp.tile([KT, btc], FP16, tag="xtail")
                nc.gpsimd.dma_start(xtl[:], xt_d[:, bsl])
                xhs = [xh_all[:, kc, :] for kc in range(NKC)]
                xls = [xl_all[:, kc, :] for kc in range(NKC)]
                xtiles[i] = (xhs, xls, xtl)

            load_bt(0)
            load_bt(1)
            # Each chain's fc2 matmul is deferred until after the NEXT
            # chain's fc1 stream, so the PE never waits on ACT's h output.
            pending = []

            def flush_pending():
                for h_t, zt_t, sl_t, final in pending:
                    ps2 = pp2.tile([M2, ns], F32, tag="ps2")
                    nc.tensor.matmul(
                        ps2[:], w2t, h_t[:], start=True, stop=True)
                    nc.vector.tensor_scalar_add(zt_t[:, sl_t], ps2[:], b2t)
                    if final is not None:
                        nc.gpsimd.dma_start(final[0], zt_t[:])
                pending.clear()

            for bt_i, btc in enumerate(bts):
                if bt_i + 2 < len(bts):
                    load_bt(bt_i + 2)  # prefetch two batch tiles ahead
                bsl = slice(offs[bt_i], offs[bt_i] + btc)
                xhs, xls, xtl = xtiles[bt_i]
                zt = zp.tile([M2, btc], F32, tag="z")
                nchains = btc // ns
                for ns_i in range(nchains):
                    sl = slice(ns_i * ns, (ns_i + 1) * ns)
                    ps1 = pp1.tile([M1, ns], F32, tag="ps1")
                    pairs = (
                        [(w1hs[kc], xhs[kc]) for kc in range(NKC)]
                        + [(w1ls[kc], xhs[kc]) for kc in range(NKC)]
                        + [(w1hs[kc], xls[kc]) for kc in range(NKC)]
                        + [(wtl, xtl)]
                    )
                    for i, (wt, xt) in enumerate(pairs):
                        nc.tensor.matmul(
                            ps1[:], wt, xt[:, sl],
                            start=(i == 0), stop=(i == len(pairs) - 1))
                    h = hp.tile([M1, ns], F32, tag="h")
                    nc.scalar.activation(
                        h[:], ps1[:], mybir.ActivationFunctionType.Relu,
                        bias=b1t)
                    flush_pending()
                    final = (z_d[:, bsl],) if ns_i == nchains - 1 else None
                    pending.append((h, zt, sl, final))
            flush_pending()
    nc.compile()
    return nc


def _fold_weights(conv_w, fc1_w):
    """Fold 3x3 valid cross-correlation + fc1 into one [128, 784] matrix."""
    cw = np.asarray(conv_w, np.float64)
    f1 = np.asarray(fc1_w, np.float64).reshape(M1, 26, 26)
    W = np.zeros((M1, 28, 28), np.float64)
    for di in range(3):
        for dj in range(3):
            W[:, di:di + 26, dj:dj + 26] += cw[di, dj] * f1
    return W.reshape(M1, K).astype(np.float32)


def _split16(a):
    hi = a.astype(np.float16)
    lo = (a.astype(np.float32) - hi.astype(np.float32)).astype(np.float16)
    return hi, lo


def kernel(x, conv_w, fc1_w, fc1_b, fc2_w, fc2_b):
    if "nc" not in _cache:
        _cache["nc"] = _build_nc()
    nc = _cache["nc"]

    w1t = np.ascontiguousarray(_fold_weights(conv_w, fc1_w).T)  # [784, 128]
    w1t_h, w1t_l = _split16(w1t)
    w_tail = np.vstack([w1t_h[KM:], w1t_l[KM:], w1t_h[KM:]])  # [48, 128]
    w2t = np.asarray(fc2_w, np.float32).T  # [128, 10]
    w_all = np.zeros((128, 1664), np.float16)
    for kc in range(NKC):
        w_all[:, kc * 128:(kc + 1) * 128] = w1t_h[kc * 128:(kc + 1) * 128, :]
        w_all[:, 768 + kc * 128: 768 + (kc + 1) * 128] = \
            w1t_l[kc * 128:(kc + 1) * 128, :]
    w_all[0:KT, 1536:1664] = w_tail
    w_all = np.ascontiguousarray(w_all)
    biases = np.zeros((M1, 12), np.float32)
    biases[:, 0] = np.asarray(fc1_b, np.float32)
    biases[0:M2, 1] = np.asarray(fc2_b, np.float32)
    biases[:, 2:12] = w2t
    x = np.asarray(x, np.float32)

    in_maps = []
    for c in range(N_CORES):
        xs = np.ascontiguousarray(x[c * B_LOCAL:(c + 1) * B_LOCAL].T)
        xh, xl = _split16(xs)
        # tail rows ordered to match w_tail: [xh_t (vs Wh), xh_t (vs Wl),
        # xl_t (vs Wh)]
        x_tail = np.ascontiguousarray(
            np.vstack([xh[KM:], xh[KM:], xl[KM:]]))  # [48, B_LOCAL]
        in_maps.append({
            "x_h": np.ascontiguousarray(xh[:KM]),
            "x_l": np.ascontiguousarray(xl[:KM]),
            "x_tail": x_tail,
            "w_all": w_all, "biases": biases,
        })
    res = run_bass_kernel_spmd(nc, in_maps, list(range(N_CORES)))
    outs = [res.results[c]["z_t"].T for c in range(N_CORES)]
    return np.ascontiguousarray(np.concatenate(outs, axis=0), dtype=np.float32)
